# revision 1
# baseline (speedup 1.0000x reference)
"""Trainium2 Bass kernel for nn_AttnModel (gnn_message_passing).

Sharding: data-parallel over graphs B=32 across 8 cores (4 graphs/core).
Only collective: AllReduce of global-layer-norm sum/sumsq of t = nf@W^T+b.

Algebra (per core; x0 = GLN(t) = alpha*(t-m)):
  x_l = a_l*x0 + B_l@V^T      (gated residual stays in span{x0, V-cols})
  x_p_l = relu(P0a_l + B_l@VU)   P0a_l = a_l*(x0@U), via P0a *= om each layer
  zx_l = g0a_l + B_l@vg          g0a_l = a_l*(gw1.x0)
  zh_l = w_l.w2v                 w_l = relu(A_l y_p)
  om = 1-z = sigmoid(-(zpre+gb));  B' = om*(B-w)+w;  P0a *= om; g0a *= om
  out = sigmoid(sum_k x_p_2 * (x_p_2 @ YPY)),  YPY_g = y_p_g^T y_p_g

Layout: "4-stacked" [128, 4096]: partition rows 32g..32g+31 = k (or j-half)
dim of graph g; free = node index within graph. blockdiag lhsT [128,128]
gives a 4-graph-parallel K=32 matmul in ONE f32r matmul (1 cyc/row).
"""

import numpy as np

B_ALL, NPG = 32, 4096
F, C, K, J = 64, 128, 32, 40
EPS = 1e-5
NC_ = 8
GPC = B_ALL // NC_            # 4 graphs per core
NLOC = GPC * NPG              # 16384 nodes per core
Q = NPG                       # 4096
CH = 512
NTOT = float(B_ALL * NPG * C)

_CACHE = {}


def _build(debug=False):
    import concourse.bass as bass
    import concourse.mybir as mybir
    import concourse.tile as tile
    import concourse.bacc as bacc

    f32 = mybir.dt.float32
    f32r = mybir.dt.float32r
    AF = mybir.ActivationFunctionType
    ALU = mybir.AluOpType
    AX = mybir.AxisListType

    nc = bacc.Bacc("TRN2", target_bir_lowering=False, debug=False, num_devices=NC_)

    def din(name, shape):
        return nc.dram_tensor(name, list(shape), f32, kind="ExternalInput")

    nf_d = din("nf", (NLOC, F))
    fragf_d = din("fragf", (B_ALL * J, F))     # full frag (stats), replicated
    fragl_d = din("fragl", (GPC * J, F))       # local 4 graphs' frag rows
    W_d = din("W_in", (C, F))
    b_d = din("b_in", (C, 1))
    U_d = din("U", (C, K))
    V_d = din("V", (C, K))
    q_d = din("q", (K, 1))
    gw1_d = din("gw1", (C, 1))
    gw2_d = din("gw2", (C, 1))
    gb_d = din("gb", (128, 1))
    id_d = din("ident", (128, 128))
    onesrow_d = din("onesrow", (1, 128))
    out_d = nc.dram_tensor("out", [GPC, NPG], f32, kind="ExternalOutput")
    dbg = {}
    def dout(name, shape):
        if debug:
            dbg[name] = nc.dram_tensor("dbg_" + name, list(shape), f32,
                                       kind="ExternalOutput")
        return dbg.get(name)

    NFR = B_ALL * J     # 1280
    NFL = GPC * J       # 160

    with tile.TileContext(nc) as tc:
        with (
            tc.tile_pool(name="const", bufs=1) as cst,
            tc.tile_pool(name="big", bufs=1) as big,
            tc.tile_pool(name="psb", bufs=1, space="PSUM") as psb,   # [128,2048]
            tc.tile_pool(name="psc", bufs=2, space="PSUM") as psc,   # [128,512]x2
            tc.tile_pool(name="psg", bufs=1, space="PSUM") as psg,   # [64,65]
            tc.tile_pool(name="dram", bufs=1, space="DRAM") as drp,
        ):
            def ctile(name, shape, dtype=f32):
                return cst.tile(list(shape), dtype, tag=name, name=name)

            def btile(name, shape, slot, dtype=f32):
                return big.tile(list(shape), dtype, tag=slot, name=name)

            def pchunk(name="pch"):
                return psc.tile([128, CH], f32, tag="pch", name=name)

            def ldc(dramt, name, shape):
                t = ctile(name, shape)
                nc.sync.dma_start(t[:], dramt.ap())
                return t

            # ---------------- constants ----------------
            tW = ldc(W_d, "W", (C, F))
            tb = ldc(b_d, "b", (C, 1))
            tU = ldc(U_d, "U", (C, K))
            tV = ldc(V_d, "V", (C, K))
            tq = ldc(q_d, "q", (K, 1))
            tgw1 = ldc(gw1_d, "gw1", (C, 1))
            tgw2 = ldc(gw2_d, "gw2", (C, 1))
            tgb = ldc(gb_d, "gb", (128, 1))
            tId = ldc(id_d, "ident", (128, 128))
            tOnesRow = ldc(onesrow_d, "onesrow", (1, 128))
            tOnes128 = ctile("ones128", (128, 1))
            nc.vector.memset(tOnes128[:], 1.0)
            tNgb = ctile("ngb", (128, 1))
            nc.vector.tensor_scalar_mul(tNgb[:], tgb[:], -1.0)
            tZero = ctile("zerof", (128, 128))
            nc.vector.memset(tZero[:], 0.0)

            def zfill(t):
                nc.vector.tensor_copy(t[:], tZero[0:t.shape[0], 0:t.shape[1]])

            tIdbd = ctile("idbd_r", (128, 128), f32r)
            tNegIdbd = ctile("negidbd_r", (128, 128), f32r)
            tSumbd = ctile("sumbd", (128, 128))
            zfill(tIdbd)
            zfill(tNegIdbd)
            nc.vector.memset(tSumbd[:], 0.0)
            for g in range(GPC):
                sl = slice(K * g, K * g + K)
                blk = tId[0:K, 0:K]
                nc.vector.tensor_copy(tIdbd[sl, sl], blk)
                nc.vector.tensor_scalar_mul(tNegIdbd[sl, sl], blk, -1.0)
                nc.vector.tensor_scalar(tSumbd[sl, sl], blk, 0.0, 1.0,
                                        ALU.mult, ALU.add)   # ones block

            # ---------------- derived weights ----------------
            rhsUg = ctile("rhsUg", (C, K + 2))
            nc.vector.tensor_copy(rhsUg[:, 0:K], tU[:])
            nc.vector.tensor_copy(rhsUg[:, K:K + 1], tgw1[:])
            nc.vector.tensor_copy(rhsUg[:, K + 1:K + 2], tgw2[:])
            pw = pchunk()
            nc.tensor.matmul(pw[0:F, 0:K + 2], tW[:], rhsUg[:], start=True, stop=True)
            tWUg = ctile("WUg", (F, K + 2))
            nc.vector.tensor_copy(tWUg[:], pw[0:F, 0:K + 2])
            pw = pchunk()
            nc.tensor.matmul(pw[0:K, 0:K + 2], tV[:], rhsUg[:], start=True, stop=True)
            tVUg = ctile("VUg", (K, K + 2))
            nc.vector.tensor_copy(tVUg[:], pw[0:K, 0:K + 2])
            pw = pchunk()
            nc.tensor.matmul(pw[0:1, 0:K + 2], tOnes128[:], rhsUg[:], start=True, stop=True)
            tColF = ctile("colF", (1, K + 2))          # [colU | sg1 | sg2] free
            nc.vector.tensor_copy(tColF[:], pw[0:1, 0:K + 2])
            pw = pchunk()
            nc.tensor.transpose(pw[0:K + 2, 0:1], tColF[:], tId[0:1, 0:1])
            tColP = ctile("colP", (K + 2, 1))
            nc.vector.tensor_copy(tColP[:], pw[0:K + 2, 0:1])
            tColUrep = ctile("colUrep", (128, 1))
            for g in range(GPC):
                nc.vector.tensor_copy(tColUrep[K * g:K * g + K, :], tColP[0:K, :])
            # lhsT for P0/g0 matmuls replicated in both partition halves
            tWUF = ctile("WUF", (128, K))
            tWg1F = ctile("wg1F", (128, K))
            for hh in (0, 1):
                nc.vector.tensor_copy(tWUF[F * hh:F * hh + F, :], tWUg[:, 0:K])
                nc.vector.tensor_copy(
                    tWg1F[F * hh:F * hh + F, :],
                    tWUg[:, K:K + 1].broadcast_to([F, K]))

            tBdVU = ctile("bdVU", (128, 128), f32r)
            tBdVG = ctile("bdVG", (128, 128), f32r)
            tBdW2V = ctile("bdW2V", (128, 128), f32r)
            zfill(tBdVU)
            zfill(tBdVG)
            zfill(tBdW2V)
            for g in range(GPC):
                sl = slice(K * g, K * g + K)
                nc.vector.tensor_copy(tBdVU[sl, sl], tVUg[:, 0:K])
                nc.vector.tensor_copy(tBdVG[sl, sl],
                                      tVUg[:, K:K + 1].broadcast_to([K, K]))
                nc.vector.tensor_copy(tBdW2V[sl, sl],
                                      tVUg[:, K + 1:K + 2].broadcast_to([K, K]))

            pw = pchunk()
            nc.tensor.transpose(pw[0:F, 0:C], tW[:], tId[:])
            tWT = ctile("WT", (F, C))
            nc.vector.tensor_copy(tWT[:], pw[0:F, 0:C])

            # ---------------- frag path ----------------
            fragT = btile("fragT", (F, NFR), "S2")
            nc.sync.dma_start_transpose(fragT[:], fragf_d.ap())
            ysT = btile("ysT", (C, NFR), "S3")
            for c0 in range(0, NFR, CH):
                w_ = min(CH, NFR - c0)
                pf = pchunk()
                nc.tensor.matmul(pf[:, 0:w_], tWT[:], fragT[:, c0:c0 + w_],
                                 start=True, stop=True)
                nc.scalar.activation(ysT[:, c0:c0 + w_], pf[:, 0:w_],
                                     AF.Identity, bias=tb[:], scale=1.0)
            fsums = ctile("fsums", (128, 5))
            nc.vector.reduce_sum(fsums[:, 0:1], ysT[:], axis=AX.X)
            for ci, c0 in enumerate(range(0, NFR, CH)):
                w_ = min(CH, NFR - c0)
                pf = pchunk()
                nc.scalar.activation(pf[:, 0:w_], ysT[:, c0:c0 + w_], AF.Square,
                                     accum_out=fsums[:, 2 + ci:3 + ci])
            nc.vector.tensor_add(fsums[:, 1:2], fsums[:, 2:3], fsums[:, 3:4])
            nc.vector.tensor_add(fsums[:, 1:2], fsums[:, 1:2], fsums[:, 4:5])
            pf = pchunk()
            nc.tensor.matmul(pf[0:2, 0:1], fsums[:, 0:2], tOnes128[:],
                             start=True, stop=True)
            fs2 = ctile("fs2", (2, 1))
            nc.vector.tensor_copy(fs2[:], pf[0:2, 0:1])
            pf2 = pchunk()
            nc.tensor.transpose(pf2[0:1, 0:2], fs2[:], tId[0:2, 0:2])
            tFS = ctile("fragstat", (1, 8))
            nc.vector.tensor_copy(tFS[:, 0:2], pf2[0:1, 0:2])
            nfr = float(C * NFR)
            nc.vector.tensor_scalar_mul(tFS[:, 2:4], tFS[:, 0:2], 1.0 / nfr)
            nc.vector.tensor_mul(tFS[:, 4:5], tFS[:, 2:3], tFS[:, 2:3])
            nc.vector.tensor_sub(tFS[:, 5:6], tFS[:, 3:4], tFS[:, 4:5])
            nc.vector.tensor_scalar_add(tFS[:, 5:6], tFS[:, 5:6], EPS)
            nc.scalar.activation(tFS[:, 6:7], tFS[:, 5:6], AF.Sqrt)
            nc.vector.reciprocal(tFS[:, 7:8], tFS[:, 6:7])                  # a2
            nc.vector.tensor_mul(tFS[:, 4:5], tFS[:, 7:8], tFS[:, 2:3])
            nc.vector.tensor_scalar_mul(tFS[:, 4:5], tFS[:, 4:5], -1.0)    # -a2*m2
            tA2c = ctile("a2c", (128, 2))
            pf = pchunk()
            nc.tensor.matmul(pf[0:128, 0:1], tOnesRow[:], tFS[:, 7:8],
                             start=True, stop=True)
            nc.tensor.matmul(pf[0:128, 1:2], tOnesRow[:], tFS[:, 4:5],
                             start=True, stop=True)
            nc.vector.tensor_copy(tA2c[:], pf[0:128, 0:2])

            # local frag -> normalized ys (f32r) -> y_p smalls
            fragTl = ctile("fragTl", (F, NFL))
            nc.sync.dma_start_transpose(fragTl[:], fragl_d.ap())
            ysTl = ctile("ysTl", (C, NFL))
            pf = pchunk()
            nc.tensor.matmul(pf[:, 0:NFL], tWT[:], fragTl[:], start=True, stop=True)
            nc.scalar.activation(ysTl[:], pf[:, 0:NFL], AF.Identity,
                                 bias=tb[:], scale=1.0)
            ysnl = ctile("ysnl", (C, NFL), f32r)
            nc.scalar.activation(ysnl[:], ysTl[:], AF.Identity,
                                 bias=tA2c[:, 1:2], scale=tA2c[:, 0:1])
            tVr = ctile("Vr", (C, K), f32r)
            nc.vector.tensor_copy(tVr[:], tV[:])
            ypT = ctile("ypT", (K, NFL))
            for g in range(GPC):
                pf = pchunk()
                nc.tensor.matmul(pf[0:K, 0:J], tVr[:], ysnl[:, J * g:J * g + J],
                                 start=True, stop=True)
                nc.scalar.activation(ypT[:, J * g:J * g + J], pf[0:K, 0:J],
                                     AF.Relu, scale=tq[:])
            tBdYT0 = ctile("bdYT0", (128, 128), f32r)
            tBdYT1 = ctile("bdYT1", (128, 128), f32r)
            zfill(tBdYT0)
            zfill(tBdYT1)
            for g in range(GPC):
                sl = slice(K * g, K * g + K)
                nc.vector.tensor_copy(tBdYT0[sl, sl], ypT[:, J * g:J * g + K])
                nc.vector.tensor_copy(tBdYT1[sl, K * g:K * g + (J - K)],
                                      ypT[:, J * g + K:J * g + J])
            ynat = ctile("ynat", (2 * K, 128))
            nc.vector.memset(ynat[:], 0.0)
            for g in range(GPC):
                pf = pchunk()
                nc.tensor.transpose(pf[0:J, 0:K], ypT[:, J * g:J * g + J],
                                    tId[0:K, 0:K])
                nc.vector.tensor_copy(ynat[0:J, K * g:K * g + K], pf[0:J, 0:K])
            tBdYPY = ctile("bdYPY", (128, 128), f32r)
            zfill(tBdYPY)
            for g in range(GPC):
                pf = pchunk()
                nc.tensor.matmul(pf[0:K, 0:K], ynat[:, K * g:K * g + K],
                                 ynat[:, K * g:K * g + K], start=True, stop=True)
                nc.vector.tensor_copy(tBdYPY[K * g:K * g + K, K * g:K * g + K],
                                      pf[0:K, 0:K])

            # ---------------- front: Gram stats + P0raw/g0raw ----------------
            P0a = btile("P0a", (128, Q), "S4")
            g0a = btile("g0a", (128, Q), "S5")
            psG = psg.tile([F, F], f32, tag="psG", name="psG")
            psS = psg.tile([F, 1], f32, tag="psS", name="psS")

            for piece in range(2):     # graphs (2*piece, 2*piece+1)
                nfT2 = btile("nfT2_%d" % piece, (128, Q), "S1")
                for gl in (0, 1):
                    g = 2 * piece + gl
                    nc.sync.dma_start_transpose(
                        nfT2[F * gl:F * gl + F, :],
                        nf_d.ap()[g * NPG:(g + 1) * NPG, :])
                for j in range(8):
                    pB = psb.tile([128, 2048], f32, tag="pI", name="pP0")
                    pP = pB[:, 0:CH]
                    pG0 = pB[:, CH:2 * CH]
                    for gl in (0, 1):
                        g = 2 * piece + gl
                        rt = F * gl
                        cols = slice(CH * j, CH * j + CH)
                        rhs = nfT2[rt:rt + F, cols]
                        pos = (rt, K * g)
                        nc.tensor.matmul(pP[K * g:K * g + K, :],
                                         tWUF[rt:rt + F, :], rhs,
                                         start=True, stop=True, tile_position=pos)
                        nc.tensor.matmul(pG0[K * g:K * g + K, :],
                                         tWg1F[rt:rt + F, :], rhs,
                                         start=True, stop=True, tile_position=pos)
                    rsl = slice(64 * piece, 64 * piece + 64)
                    dst = slice(CH * j, CH * j + CH)
                    nc.scalar.activation(P0a[rsl, dst], pP[rsl, :], AF.Identity)
                    nc.vector.tensor_copy(g0a[rsl, dst], pG0[rsl, :])
                # Gram over this piece's nodes (natural layout)
                half = btile("nfnat_%d" % piece, (128, Q), "S2")
                src = nf_d.ap()[piece * (NLOC // 2):(piece + 1) * (NLOC // 2), :]
                nc.sync.dma_start(half[:].rearrange("p (b f) -> p b f", b=64),
                                  src.rearrange("(b p) f -> p b f", p=128))
                for b in range(64):
                    st = (piece == 0 and b == 0)
                    sp = (piece == 1 and b == 63)
                    tile_b = half[:, F * b:F * b + F]
                    nc.tensor.matmul(psG[:], tile_b, tile_b,
                                     start=st, stop=sp, skip_group_check=True)
                    nc.tensor.matmul(psS[:], tile_b, tOnes128[:],
                                     start=st, stop=sp, skip_group_check=True)

            tGs = ctile("Gs", (F, F + 1))
            nc.vector.tensor_copy(tGs[:, 0:F], psG[:])
            nc.vector.tensor_copy(tGs[:, F:F + 1], psS[:])
            pf = pchunk()
            nc.tensor.matmul(pf[0:F, 0:C], tGs[:, 0:F], tWT[:], start=True, stop=True)
            tGW = ctile("GW", (F, C))
            nc.vector.tensor_mul(tGW[:], pf[0:F, 0:C], tWT[:])
            pf = pchunk()
            nc.tensor.matmul(pf[0:C, 0:1], tGW[:], tOnes128[0:F, :],
                             start=True, stop=True)              # quad_c
            nc.tensor.matmul(pf[0:C, 1:2], tWT[:], tGs[:, F:F + 1],
                             start=True, stop=True)              # ws_c
            tM5 = ctile("M5", (128, 5))
            nc.vector.tensor_copy(tM5[:, 0:2], pf[0:C, 0:2])
            nc.vector.tensor_copy(tM5[:, 2:3], tb[:])
            nc.vector.tensor_mul(tM5[:, 3:4], tb[:], tb[:])
            nc.vector.tensor_mul(tM5[:, 4:5], tb[:], tM5[:, 1:2])
            pf = pchunk()
            nc.tensor.matmul(pf[0:5, 0:1], tM5[:], tOnes128[:], start=True, stop=True)
            st5 = ctile("st5", (5, 1))
            nc.vector.tensor_copy(st5[:], pf[0:5, 0:1])
            pf2 = pchunk()
            nc.tensor.transpose(pf2[0:1, 0:5], st5[:], tId[0:5, 0:5])
            tST = ctile("stat", (1, 12))
            nc.vector.tensor_copy(tST[:, 0:5], pf2[0:1, 0:5])
            # [0]=quad [1]=ws [2]=b [3]=b2 [4]=bws
            nc.vector.tensor_scalar(tST[:, 5:6], tST[:, 2:3], float(NLOC), None,
                                    ALU.mult)
            nc.vector.tensor_add(tST[:, 5:6], tST[:, 5:6], tST[:, 1:2])
            nc.vector.tensor_scalar(tST[:, 6:7], tST[:, 4:5], 2.0, None, ALU.mult)
            nc.vector.tensor_add(tST[:, 6:7], tST[:, 6:7], tST[:, 0:1])
            nc.vector.tensor_scalar(tST[:, 7:8], tST[:, 3:4], float(NLOC), None,
                                    ALU.mult)
            nc.vector.tensor_add(tST[:, 6:7], tST[:, 6:7], tST[:, 7:8])

            # ---- collective: AllReduce [sum, sumsq] ----
            cin = ctile("cin", (1, 128))
            nc.vector.memset(cin[:], 0.0)
            nc.vector.tensor_copy(cin[:, 0:1], tST[:, 5:6])
            nc.vector.tensor_copy(cin[:, 1:2], tST[:, 6:7])
            db_in = drp.tile([1, 128], f32, name="db_in")
            db_out = drp.tile([1, 128], f32, name="db_out")
            nc.sync.dma_start(db_in[:], cin[:])
            nc.gpsimd.collective_compute(
                "AllReduce", mybir.AluOpType.add,
                replica_groups=[list(range(NC_))],
                ins=[db_in.opt()], outs=[db_out.opt()],
            )
            cout = ctile("cout", (1, 128))
            nc.sync.dma_start(cout[:], db_out[:])
            tGS = ctile("gstat", (1, 8))
            nc.vector.tensor_scalar_mul(tGS[:, 0:2], cout[:, 0:2], 1.0 / NTOT)
            nc.vector.tensor_mul(tGS[:, 2:3], tGS[:, 0:1], tGS[:, 0:1])
            nc.vector.tensor_sub(tGS[:, 2:3], tGS[:, 1:2], tGS[:, 2:3])
            nc.vector.tensor_scalar_add(tGS[:, 2:3], tGS[:, 2:3], EPS)
            nc.scalar.activation(tGS[:, 3:4], tGS[:, 2:3], AF.Sqrt)
            nc.vector.reciprocal(tGS[:, 4:5], tGS[:, 3:4])              # alpha
            nc.vector.tensor_mul(tGS[:, 5:6], tGS[:, 4:5], tGS[:, 0:1])
            nc.vector.tensor_scalar_mul(tGS[:, 5:6], tGS[:, 5:6], -1.0)  # -am
            nc.vector.tensor_mul(tGS[:, 6:7], tGS[:, 5:6], tColF[:, K:K + 1])
            tAB = ctile("alphab", (128, 3))
            pf = pchunk()
            for ii, cidx in [(0, 4), (1, 5), (2, 6)]:
                nc.tensor.matmul(pf[0:128, ii:ii + 1], tOnesRow[:],
                                 tGS[:, cidx:cidx + 1], start=True, stop=True)
            nc.vector.tensor_copy(tAB[:], pf[0:128, 0:3])
            tBiasP0 = ctile("biasP0", (128, 1))
            nc.vector.tensor_mul(tBiasP0[:], tColUrep[:], tAB[:, 1:2])
            # normalize in place: P0a = alpha*P0raw + biasP0 ; g0a likewise
            nc.scalar.activation(P0a[:], P0a[:], AF.Identity,
                                 bias=tBiasP0[:], scale=tAB[:, 0:1])
            nc.scalar.activation(g0a[:], g0a[:], AF.Identity,
                                 bias=tAB[:, 2:3], scale=tAB[:, 0:1])
            if debug:
                d = dout("Gs", (F, F + 1)); nc.sync.dma_start(d.ap(), tGs[:])
                d = dout("st", (1, 8)); nc.sync.dma_start(d.ap(), tST[:, 0:8])
                d = dout("P0a", (128, Q)); nc.sync.dma_start(d.ap(), P0a[:])
                d = dout("g0a", (128, Q)); nc.sync.dma_start(d.ap(), g0a[:])
                d = dout("gstat", (1, 7)); nc.sync.dma_start(d.ap(), tGS[:, 0:7])
            xp = btile("xp", (128, Q), "S6", f32r)
            nc.scalar.activation(xp[:], P0a[:], AF.Relu)
            if debug:
                d = dout("xp0", (128, Q)); nc.sync.dma_start(d.ap(), xp[:].bitcast(f32))

            # ---------------- layers 0,1 ----------------
            Bst = None
            for l in range(2):
                bdYT = [tBdYT0, tBdYT1]
                mx = ctile("mx%d" % l, (128, 12))
                # pass 1: I^T fills, row maxes
                for h in range(2):
                    for hq in range(2):
                        pI = psb.tile([128, 2048], f32, tag="pI", name="pI")
                        for cc in range(4):
                            cs = slice(2048 * hq + CH * cc, 2048 * hq + CH * cc + CH)
                            nc.tensor.matmul(pI[:, CH * cc:CH * cc + CH],
                                             bdYT[h][:], xp[:, cs],
                                             start=True, stop=True)
                        nc.vector.reduce_max(mx[:, 4 * h + hq:4 * h + hq + 1],
                                             pI[:], axis=AX.X)
                    nc.vector.tensor_max(mx[:, 4 * h + 2:4 * h + 3],
                                         mx[:, 4 * h:4 * h + 1],
                                         mx[:, 4 * h + 1:4 * h + 2])
                    nc.vector.tensor_scalar_mul(mx[:, 4 * h + 3:4 * h + 4],
                                                mx[:, 4 * h + 2:4 * h + 3], -1.0)
                # pass 2: regenerate I^T, exp -> E, accumulate S
                E = btile("E%d" % l, (128, 2 * Q), "S1", f32r)
                S = ctile("S%d" % l, (128, 6))
                for h in range(2):
                    for hq in range(2):
                        pI = psb.tile([128, 2048], f32, tag="pI", name="pI")
                        for cc in range(4):
                            cs = slice(2048 * hq + CH * cc, 2048 * hq + CH * cc + CH)
                            nc.tensor.matmul(pI[:, CH * cc:CH * cc + CH],
                                             bdYT[h][:], xp[:, cs],
                                             start=True, stop=True)
                        nc.scalar.activation(
                            E[:, Q * h + 2048 * hq:Q * h + 2048 * hq + 2048],
                            pI[:], AF.Exp, bias=mx[:, 4 * h + 3:4 * h + 4],
                            accum_out=S[:, 2 * h + hq:2 * h + hq + 1])
                rS = ctile("rS%d" % l, (128, 2))
                for h in range(2):
                    nc.vector.tensor_add(S[:, 4 + h:5 + h], S[:, 2 * h:2 * h + 1],
                                         S[:, 2 * h + 1:2 * h + 2])
                    nc.vector.reciprocal(rS[:, h:h + 1], S[:, 4 + h:5 + h])
                bdYtil0 = ctile("bdYtil0_%d" % l, (128, 128), f32r)
                bdYtil1 = ctile("bdYtil1_%d" % l, (128, 128), f32r)
                zfill(bdYtil0)
                zfill(bdYtil1)
                for g in range(GPC):
                    sl = slice(K * g, K * g + K)
                    nc.vector.tensor_scalar(bdYtil0[sl, sl], ynat[0:K, sl],
                                            rS[sl, 0:1], None, ALU.mult)
                    nc.vector.tensor_scalar(bdYtil1[sl, sl], ynat[K:2 * K, sl],
                                            rS[sl, 1:2], None, ALU.mult)
                # w = relu(A y_p)
                wt = btile("w%d" % l, (128, Q), "S7", f32r)
                for cc in range(8):
                    pW = pchunk("pW")
                    nc.tensor.matmul(pW[:], bdYtil0[:], E[:, CH * cc:CH * cc + CH],
                                     start=True, stop=False)
                    nc.tensor.matmul(pW[:], bdYtil1[:],
                                     E[:, Q + CH * cc:Q + CH * cc + CH],
                                     start=False, stop=True)
                    nc.scalar.activation(wt[:, CH * cc:CH * cc + CH], pW[:], AF.Relu)
                # zpre = zh + B@vg + g0a ; om = sigmoid(-(zpre+gb))
                zs = btile("zs%d" % l, (128, Q), "S2")
                for cc in range(8):
                    pZ = pchunk("pZ")
                    nc.tensor.matmul(pZ[:], tBdW2V[:], wt[:, CH * cc:CH * cc + CH],
                                     start=True, stop=(Bst is None))
                    if Bst is not None:
                        nc.tensor.matmul(pZ[:], tBdVG[:], Bst[:, CH * cc:CH * cc + CH],
                                         start=False, stop=True)
                    nc.vector.tensor_add(zs[:, CH * cc:CH * cc + CH], pZ[:],
                                         g0a[:, CH * cc:CH * cc + CH])
                om = btile("om%d" % l, (128, Q), "S8")
                nc.scalar.activation(om[:], zs[:], AF.Sigmoid,
                                     bias=tNgb[:], scale=-1.0)
                if debug:
                    d = dout("w%d" % l, (128, Q)); nc.sync.dma_start(d.ap(), wt[:].bitcast(f32))
                    d = dout("om%d" % l, (128, Q)); nc.sync.dma_start(d.ap(), om[:])
                    d = dout("S%d" % l, (128, 6)); nc.sync.dma_start(d.ap(), S[:])
                    d = dout("mx%d" % l, (128, 8)); nc.sync.dma_start(d.ap(), mx[:, 0:8])
                # B' = om*(B - w) + w   (B=0 at l=0 -> B1 = w - om*w)
                if Bst is None:
                    m1 = btile("m1", (128, Q), "S2")
                    nc.vector.tensor_mul(m1[:], om[:], wt[:])
                    B1 = btile("B1", (128, Q), "S3", f32r)
                    nc.vector.tensor_sub(B1[:], wt[:], m1[:])
                    Bst = B1
                else:
                    dd = btile("dd", (128, Q), "S2")
                    nc.vector.tensor_sub(dd[:], Bst[:], wt[:])
                    nc.vector.tensor_mul(dd[:], dd[:], om[:])
                    B2 = btile("B2", (128, Q), "S1", f32r)   # E is dead by now
                    nc.vector.tensor_add(B2[:], wt[:], dd[:])
                    Bst = B2
                # P0a *= om ; g0a *= om (g0a only needed for next layer's gate)
                nc.vector.tensor_mul(P0a[:], P0a[:], om[:])
                if l == 0:
                    nc.vector.tensor_mul(g0a[:], g0a[:], om[:])
                # x_p for layer l+1 = relu(P0a + B@VU)
                xpre = btile("xpre%d" % l, (128, Q), "S2")
                for cc in range(8):
                    pX = pchunk("pX")
                    nc.tensor.matmul(pX[:], tBdVU[:], Bst[:, CH * cc:CH * cc + CH],
                                     start=True, stop=True)
                    nc.vector.tensor_add(xpre[:, CH * cc:CH * cc + CH], pX[:],
                                         P0a[:, CH * cc:CH * cc + CH])
                xp = btile("xp_%d" % (l + 1), (128, Q), "S6", f32r)
                nc.scalar.activation(xp[:], xpre[:], AF.Relu)
                if debug:
                    d = dout("xp%d" % (l + 1), (128, Q))
                    nc.sync.dma_start(d.ap(), xp[:].bitcast(f32))

            # ---------------- final ----------------
            tmp = btile("tmp", (128, Q), "S2")
            for cc in range(8):
                pP = pchunk("pF")
                nc.tensor.matmul(pP[:], tBdYPY[:], xp[:, CH * cc:CH * cc + CH],
                                 start=True, stop=True)
                nc.vector.tensor_mul(tmp[:, CH * cc:CH * cc + CH], pP[:],
                                     xp[:, CH * cc:CH * cc + CH])
            sOut = btile("sOut", (128, Q), "S4")     # P0a dead now
            for cc in range(8):
                pS = pchunk("pS")
                nc.tensor.matmul(pS[:], tSumbd[:], tmp[:, CH * cc:CH * cc + CH],
                                 start=True, stop=True)
                nc.scalar.activation(sOut[:, CH * cc:CH * cc + CH], pS[:], AF.Sigmoid)
            for g in range(GPC):
                nc.sync.dma_start(out_d.ap()[g:g + 1, :],
                                  sOut[K * g:K * g + 1, :])

    nc.compile()
    return nc


def _get_program(debug=False):
    key = "nc_dbg" if debug else "nc"
    if key not in _CACHE:
        _CACHE[key] = _build(debug)
    return _CACHE[key]


def make_in_maps(inputs):
    nf = np.ascontiguousarray(np.asarray(inputs["node_feats"], np.float32))
    frag = np.ascontiguousarray(
        np.asarray(inputs["frag_emb"], np.float32).reshape(B_ALL * J, F))
    W = np.ascontiguousarray(np.asarray(inputs["W_in"], np.float32))
    b = np.asarray(inputs["b_in"], np.float32).reshape(C, 1)
    U = np.ascontiguousarray(np.asarray(inputs["U"], np.float32))
    V = np.ascontiguousarray(np.asarray(inputs["V"], np.float32))
    q = np.asarray(inputs["q"], np.float32).reshape(K, 1)
    gW = np.asarray(inputs["gate_W"], np.float32).reshape(2 * C)
    gb = np.asarray(inputs["gate_b"], np.float32).reshape(1)
    in_maps = []
    for c in range(NC_):
        in_maps.append({
            "nf": nf[c * NLOC:(c + 1) * NLOC],
            "fragf": frag,
            "fragl": np.ascontiguousarray(frag[c * GPC * J:(c + 1) * GPC * J]),
            "W_in": W, "b_in": b, "U": U, "V": V, "q": q,
            "gw1": np.ascontiguousarray(gW[:C].reshape(C, 1)),
            "gw2": np.ascontiguousarray(gW[C:].reshape(C, 1)),
            "gb": np.full((128, 1), gb[0], np.float32),
            "ident": np.eye(128, dtype=np.float32),
            "onesrow": np.ones((1, 128), np.float32),
        })
    return in_maps


def kernel(**inputs):
    from concourse.bass_utils import run_bass_kernel_spmd

    nc = _get_program()
    in_maps = make_in_maps(inputs)
    res = run_bass_kernel_spmd(nc, in_maps, core_ids=list(range(NC_)))
    return np.concatenate([r["out"] for r in res.results], axis=0)



# revision 7
# speedup vs baseline: 4.3915x; 4.3915x over previous
"""Trainium2 Bass kernel for nn_AttnModel (gnn_message_passing).

Sharding: data-parallel over graphs B=32 across 8 cores (4 graphs/core).
Only collective: AllReduce of global-layer-norm sum/sumsq of t = nf@W^T+b.

Algebra (per core; x0 = GLN(t) = alpha*(t-m)):
  x_l = a_l*x0 + B_l@V^T      (gated residual stays in span{x0, V-cols})
  x_p_l = relu(P0a_l + B_l@VU)   P0a_l = a_l*(x0@U), via P0a *= om each layer
  zx_l = g0a_l + B_l@vg          g0a_l = a_l*(gw1.x0)
  zh_l = w_l.w2v                 w_l = relu(A_l y_p)
  om = 1-z = sigmoid(-(zpre+gb));  B' = om*(B-w)+w;  P0a *= om; g0a *= om
  out = sigmoid(sum_k x_p_2 * (x_p_2 @ YPY)),  YPY_g = y_p_g^T y_p_g

Layout: "4-stacked" [128, 4096]: partition rows 32g..32g+31 = k (or j-half)
dim of graph g; free = node index within graph. blockdiag lhsT [128,128]
gives a 4-graph-parallel K=32 matmul in ONE f32r matmul (1 cyc/row).
"""

import numpy as np

B_ALL, NPG = 32, 4096
F, C, K, J = 64, 128, 32, 40
EPS = 1e-5
NC_ = 8
GPC = B_ALL // NC_            # 4 graphs per core
NLOC = GPC * NPG              # 16384 nodes per core
Q = NPG                       # 4096
CH = 512
NTOT = float(B_ALL * NPG * C)

_CACHE = {}


def _build(debug=False):
    import concourse.bass as bass
    import concourse.mybir as mybir
    import concourse.tile as tile
    import concourse.bacc as bacc

    f32 = mybir.dt.float32
    f32r = mybir.dt.float32r
    AF = mybir.ActivationFunctionType
    ALU = mybir.AluOpType
    AX = mybir.AxisListType

    nc = bacc.Bacc("TRN2", target_bir_lowering=False, debug=False, num_devices=NC_)

    def din(name, shape):
        return nc.dram_tensor(name, list(shape), f32, kind="ExternalInput")

    # Host-prepacked layouts (dense, per-partition contiguous DMA):
    #   nfT:  [2*128, 4096]  piece p rows 128p..: [g=2p feats 0-63 | g=2p+1]^T
    #   nf65: [2*128, 64*65] piece p: 64 node-blocks of [128 nodes, 64 f + ones]
    nfT_d = din("nfT", (2 * 128, Q))
    nf65_d = din("nf65", (2 * 128, 64 * (F + 1)))
    fragT_d = din("fragT", (F, B_ALL * J))     # full frag^T (stats), replicated
    fragTl_d = din("fragTl", (F, GPC * J))     # local 4 graphs' frag^T
    W_d = din("W_in", (C, F))
    b_d = din("b_in", (C, 1))
    U_d = din("U", (C, K))
    V_d = din("V", (C, K))
    q_d = din("q", (K, 1))
    gw1_d = din("gw1", (C, 1))
    gw2_d = din("gw2", (C, 1))
    gb_d = din("gb", (128, 1))
    id_d = din("ident", (128, 128))
    onesrow_d = din("onesrow", (1, 128))
    out_d = nc.dram_tensor("out", [GPC, NPG], f32, kind="ExternalOutput")
    dbg = {}
    def dout(name, shape):
        if debug:
            dbg[name] = nc.dram_tensor("dbg_" + name, list(shape), f32,
                                       kind="ExternalOutput")
        return dbg.get(name)

    NFR = B_ALL * J     # 1280
    NFL = GPC * J       # 160

    with tile.TileContext(nc) as tc:
        with (
            tc.tile_pool(name="const", bufs=1) as cst,
            tc.tile_pool(name="big", bufs=1) as big,
            tc.tile_pool(name="psb", bufs=1, space="PSUM") as psb,   # [128,2048]
            tc.tile_pool(name="psc", bufs=2, space="PSUM") as psc,   # [128,512]x2
            tc.tile_pool(name="psg", bufs=1, space="PSUM") as psg,   # [64,65]
            tc.tile_pool(name="dram", bufs=1, space="DRAM") as drp,
        ):
            def ctile(name, shape, dtype=f32):
                return cst.tile(list(shape), dtype, tag=name, name=name)

            def btile(name, shape, slot, dtype=f32):
                return big.tile(list(shape), dtype, tag=slot, name=name)

            def pchunk(name="pch"):
                return psc.tile([128, CH], f32, tag="pch", name=name)

            def ldc(dramt, name, shape):
                t = ctile(name, shape)
                nc.sync.dma_start(t[:], dramt.ap())
                return t

            # ---------------- constants ----------------
            tW = ldc(W_d, "W", (C, F))
            tb = ldc(b_d, "b", (C, 1))
            tU = ldc(U_d, "U", (C, K))
            tV = ldc(V_d, "V", (C, K))
            tq = ldc(q_d, "q", (K, 1))
            tgw1 = ldc(gw1_d, "gw1", (C, 1))
            tgw2 = ldc(gw2_d, "gw2", (C, 1))
            tgb = ldc(gb_d, "gb", (128, 1))
            tId = ldc(id_d, "ident", (128, 128))
            tOnesRow = ldc(onesrow_d, "onesrow", (1, 128))
            tOnes128 = ctile("ones128", (128, 1))
            nc.vector.memset(tOnes128[:], 1.0)
            tNgb = ctile("ngb", (128, 1))
            nc.vector.tensor_scalar_mul(tNgb[:], tgb[:], -1.0)
            tZero = ctile("zerof", (128, 128))
            nc.vector.memset(tZero[:], 0.0)

            def zfill(t):
                nc.vector.tensor_copy(t[:], tZero[0:t.shape[0], 0:t.shape[1]])

            tIdbd = ctile("idbd_r", (128, 128), f32r)
            tNegIdbd = ctile("negidbd_r", (128, 128), f32r)
            tSumbd = ctile("sumbd", (128, 128))
            zfill(tIdbd)
            zfill(tNegIdbd)
            nc.vector.memset(tSumbd[:], 0.0)
            for g in range(GPC):
                sl = slice(K * g, K * g + K)
                blk = tId[0:K, 0:K]
                nc.vector.tensor_copy(tIdbd[sl, sl], blk)
                nc.vector.tensor_scalar_mul(tNegIdbd[sl, sl], blk, -1.0)
                nc.vector.tensor_scalar(tSumbd[sl, sl], blk, 0.0, 1.0,
                                        ALU.mult, ALU.add)   # ones block

            # ---------------- derived weights ----------------
            rhsUg = ctile("rhsUg", (C, K + 2))
            nc.vector.tensor_copy(rhsUg[:, 0:K], tU[:])
            nc.vector.tensor_copy(rhsUg[:, K:K + 1], tgw1[:])
            nc.vector.tensor_copy(rhsUg[:, K + 1:K + 2], tgw2[:])
            pw = pchunk()
            nc.tensor.matmul(pw[0:F, 0:K + 2], tW[:], rhsUg[:], start=True, stop=True)
            tWUg = ctile("WUg", (F, K + 2))
            nc.vector.tensor_copy(tWUg[:], pw[0:F, 0:K + 2])
            pw = pchunk()
            nc.tensor.matmul(pw[0:K, 0:K + 2], tV[:], rhsUg[:], start=True, stop=True)
            tVUg = ctile("VUg", (K, K + 2))
            nc.vector.tensor_copy(tVUg[:], pw[0:K, 0:K + 2])
            pw = pchunk()
            nc.tensor.matmul(pw[0:1, 0:K + 2], tOnes128[:], rhsUg[:], start=True, stop=True)
            tColF = ctile("colF", (1, K + 2))          # [colU | sg1 | sg2] free
            nc.vector.tensor_copy(tColF[:], pw[0:1, 0:K + 2])
            pw = pchunk()
            nc.tensor.transpose(pw[0:K + 2, 0:1], tColF[:], tId[0:1, 0:1])
            tColP = ctile("colP", (K + 2, 1))
            nc.vector.tensor_copy(tColP[:], pw[0:K + 2, 0:1])
            tColUrep = ctile("colUrep", (128, 1))
            for g in range(GPC):
                nc.vector.tensor_copy(tColUrep[K * g:K * g + K, :], tColP[0:K, :])
            # lhsT for P0/g0 matmuls replicated in both partition halves
            tWUF = ctile("WUF", (128, K))
            tWg1F = ctile("wg1F", (128, K))
            for hh in (0, 1):
                nc.vector.tensor_copy(tWUF[F * hh:F * hh + F, :], tWUg[:, 0:K])
                nc.vector.tensor_copy(
                    tWg1F[F * hh:F * hh + F, :],
                    tWUg[:, K:K + 1].broadcast_to([F, K]))

            tBdVU = ctile("bdVU", (128, 128), f32r)
            tBdVG = ctile("bdVG", (128, 128), f32r)
            tBdW2V = ctile("bdW2V", (128, 128), f32r)
            zfill(tBdVU)
            zfill(tBdVG)
            zfill(tBdW2V)
            for g in range(GPC):
                sl = slice(K * g, K * g + K)
                nc.vector.tensor_copy(tBdVU[sl, sl], tVUg[:, 0:K])
                nc.vector.tensor_copy(tBdVG[sl, sl],
                                      tVUg[:, K:K + 1].broadcast_to([K, K]))
                nc.vector.tensor_copy(tBdW2V[sl, sl],
                                      tVUg[:, K + 1:K + 2].broadcast_to([K, K]))

            pw = pchunk()
            nc.tensor.transpose(pw[0:F, 0:C], tW[:], tId[:])
            tWT = ctile("WT", (F, C))
            nc.vector.tensor_copy(tWT[:], pw[0:F, 0:C])

            # ---------------- Gram + global stats (first: gates collective) ----
            nf65_0 = big.tile([128, 64 * (F + 1)], f32, tag="S1", name="nf65_0")
            nf65_1 = big.tile([128, 64 * (F + 1)], f32, tag="GR1", name="nf65_1")
            nc.sync.dma_start(nf65_0[:], nf65_d.ap()[0:128, :])
            nc.sync.dma_start(nf65_1[:], nf65_d.ap()[128:256, :])
            psG = psg.tile([F + 1, F + 1], f32, tag="psG", name="psG")
            for piece, t65 in ((0, nf65_0), (1, nf65_1)):
                for b in range(64):
                    st = (piece == 0 and b == 0)
                    sp = (piece == 1 and b == 63)
                    blk = t65[:, (F + 1) * b:(F + 1) * b + (F + 1)]
                    nc.tensor.matmul(psG[:], blk, blk,
                                     start=st, stop=sp, skip_group_check=True)

            tGs = ctile("Gs", (F, F + 1))
            nc.vector.tensor_copy(tGs[:, 0:F], psG[0:F, 0:F])
            nc.vector.tensor_copy(tGs[:, F:F + 1], psG[0:F, F:F + 1])
            pf = pchunk()
            nc.tensor.matmul(pf[0:F, 0:C], tGs[:, 0:F], tWT[:],
                             start=True, stop=True)
            tGW = ctile("GW", (F, C))
            nc.vector.tensor_mul(tGW[:], pf[0:F, 0:C], tWT[:])
            pf = pchunk()
            nc.tensor.matmul(pf[0:C, 0:1], tGW[:], tOnes128[0:F, :],
                             start=True, stop=True)              # quad_c
            nc.tensor.matmul(pf[0:C, 1:2], tWT[:], tGs[:, F:F + 1],
                             start=True, stop=True)              # ws_c
            tM5 = ctile("M5", (128, 5))
            nc.vector.tensor_copy(tM5[:, 0:2], pf[0:C, 0:2])
            nc.vector.tensor_copy(tM5[:, 2:3], tb[:])
            nc.vector.tensor_mul(tM5[:, 3:4], tb[:], tb[:])
            nc.vector.tensor_mul(tM5[:, 4:5], tb[:], tM5[:, 1:2])
            pf = pchunk()
            nc.tensor.matmul(pf[0:5, 0:1], tM5[:], tOnes128[:], start=True, stop=True)
            st5 = ctile("st5", (5, 1))
            nc.vector.tensor_copy(st5[:], pf[0:5, 0:1])
            pf2 = pchunk()
            nc.tensor.transpose(pf2[0:1, 0:5], st5[:], tId[0:5, 0:5])
            tST = ctile("stat", (1, 12))
            nc.vector.tensor_copy(tST[:, 0:5], pf2[0:1, 0:5])
            # [0]=quad [1]=ws [2]=b [3]=b2 [4]=bws
            nc.vector.tensor_scalar(tST[:, 5:6], tST[:, 2:3], float(NLOC), None,
                                    ALU.mult)
            nc.vector.tensor_add(tST[:, 5:6], tST[:, 5:6], tST[:, 1:2])
            nc.vector.tensor_scalar(tST[:, 6:7], tST[:, 4:5], 2.0, None, ALU.mult)
            nc.vector.tensor_add(tST[:, 6:7], tST[:, 6:7], tST[:, 0:1])
            nc.vector.tensor_scalar(tST[:, 7:8], tST[:, 3:4], float(NLOC), None,
                                    ALU.mult)
            nc.vector.tensor_add(tST[:, 6:7], tST[:, 6:7], tST[:, 7:8])

            # ---- collective: AllReduce [sum, sumsq] (flies during front) ----
            cin = ctile("cin", (1, 128))
            nc.vector.memset(cin[:], 0.0)
            nc.vector.tensor_copy(cin[:, 0:1], tST[:, 5:6])
            nc.vector.tensor_copy(cin[:, 1:2], tST[:, 6:7])
            db_in = drp.tile([1, 128], f32, name="db_in")
            db_out = drp.tile([1, 128], f32, name="db_out")
            nc.sync.dma_start(db_in[:], cin[:])
            nc.gpsimd.collective_compute(
                "AllReduce", mybir.AluOpType.add,
                replica_groups=[list(range(NC_))],
                ins=[db_in.opt()], outs=[db_out.opt()],
            )
            cout = ctile("cout", (1, 128))
            nc.sync.dma_start(cout[:], db_out[:])

            # ---------------- frag path ----------------
            fragT = btile("fragT", (F, NFR), "S2")
            nc.sync.dma_start(fragT[:], fragT_d.ap())
            ysT = btile("ysT", (C, NFR), "S3")
            for c0 in range(0, NFR, CH):
                w_ = min(CH, NFR - c0)
                pf = pchunk()
                nc.tensor.matmul(pf[:, 0:w_], tWT[:], fragT[:, c0:c0 + w_],
                                 start=True, stop=True)
                nc.scalar.activation(ysT[:, c0:c0 + w_], pf[:, 0:w_],
                                     AF.Identity, bias=tb[:], scale=1.0)
            fsums = ctile("fsums", (128, 5))
            nc.vector.reduce_sum(fsums[:, 0:1], ysT[:], axis=AX.X)
            for ci, c0 in enumerate(range(0, NFR, CH)):
                w_ = min(CH, NFR - c0)
                pf = pchunk()
                nc.scalar.activation(pf[:, 0:w_], ysT[:, c0:c0 + w_], AF.Square,
                                     accum_out=fsums[:, 2 + ci:3 + ci])
            nc.vector.tensor_add(fsums[:, 1:2], fsums[:, 2:3], fsums[:, 3:4])
            nc.vector.tensor_add(fsums[:, 1:2], fsums[:, 1:2], fsums[:, 4:5])
            pf = pchunk()
            nc.tensor.matmul(pf[0:2, 0:1], fsums[:, 0:2], tOnes128[:],
                             start=True, stop=True)
            fs2 = ctile("fs2", (2, 1))
            nc.vector.tensor_copy(fs2[:], pf[0:2, 0:1])
            pf2 = pchunk()
            nc.tensor.transpose(pf2[0:1, 0:2], fs2[:], tId[0:2, 0:2])
            tFS = ctile("fragstat", (1, 8))
            nc.vector.tensor_copy(tFS[:, 0:2], pf2[0:1, 0:2])
            nfr = float(C * NFR)
            nc.vector.tensor_scalar_mul(tFS[:, 2:4], tFS[:, 0:2], 1.0 / nfr)
            nc.vector.tensor_mul(tFS[:, 4:5], tFS[:, 2:3], tFS[:, 2:3])
            nc.vector.tensor_sub(tFS[:, 5:6], tFS[:, 3:4], tFS[:, 4:5])
            nc.vector.tensor_scalar_add(tFS[:, 5:6], tFS[:, 5:6], EPS)
            nc.scalar.activation(tFS[:, 6:7], tFS[:, 5:6], AF.Sqrt)
            nc.vector.reciprocal(tFS[:, 7:8], tFS[:, 6:7])                  # a2
            nc.vector.tensor_mul(tFS[:, 4:5], tFS[:, 7:8], tFS[:, 2:3])
            nc.vector.tensor_scalar_mul(tFS[:, 4:5], tFS[:, 4:5], -1.0)    # -a2*m2
            tA2c = ctile("a2c", (128, 2))
            pf = pchunk()
            nc.tensor.matmul(pf[0:128, 0:1], tOnesRow[:], tFS[:, 7:8],
                             start=True, stop=True)
            nc.tensor.matmul(pf[0:128, 1:2], tOnesRow[:], tFS[:, 4:5],
                             start=True, stop=True)
            nc.vector.tensor_copy(tA2c[:], pf[0:128, 0:2])

            # local frag -> normalized ys (f32r) -> y_p smalls
            fragTl = ctile("fragTl", (F, NFL))
            nc.sync.dma_start(fragTl[:], fragTl_d.ap())
            ysTl = ctile("ysTl", (C, NFL))
            pf = pchunk()
            nc.tensor.matmul(pf[:, 0:NFL], tWT[:], fragTl[:], start=True, stop=True)
            nc.scalar.activation(ysTl[:], pf[:, 0:NFL], AF.Identity,
                                 bias=tb[:], scale=1.0)
            ysnl = ctile("ysnl", (C, NFL), f32r)
            nc.scalar.activation(ysnl[:], ysTl[:], AF.Identity,
                                 bias=tA2c[:, 1:2], scale=tA2c[:, 0:1])
            tVr = ctile("Vr", (C, K), f32r)
            nc.vector.tensor_copy(tVr[:], tV[:])
            ypT = ctile("ypT", (K, NFL))
            for g in range(GPC):
                pf = pchunk()
                nc.tensor.matmul(pf[0:K, 0:J], tVr[:], ysnl[:, J * g:J * g + J],
                                 start=True, stop=True)
                nc.scalar.activation(ypT[:, J * g:J * g + J], pf[0:K, 0:J],
                                     AF.Relu, scale=tq[:])
            tBdYT0 = ctile("bdYT0", (128, 128), f32r)
            tBdYT1 = ctile("bdYT1", (128, 128), f32r)
            zfill(tBdYT0)
            zfill(tBdYT1)
            for g in range(GPC):
                sl = slice(K * g, K * g + K)
                nc.vector.tensor_copy(tBdYT0[sl, sl], ypT[:, J * g:J * g + K])
                nc.vector.tensor_copy(tBdYT1[sl, K * g:K * g + (J - K)],
                                      ypT[:, J * g + K:J * g + J])
            ynat = ctile("ynat", (2 * K, 128))
            nc.vector.memset(ynat[:], 0.0)
            for g in range(GPC):
                pf = pchunk()
                nc.tensor.transpose(pf[0:J, 0:K], ypT[:, J * g:J * g + J],
                                    tId[0:K, 0:K])
                nc.vector.tensor_copy(ynat[0:J, K * g:K * g + K], pf[0:J, 0:K])
            tBdYPY = ctile("bdYPY", (128, 128), f32r)
            zfill(tBdYPY)
            for g in range(GPC):
                pf = pchunk()
                nc.tensor.matmul(pf[0:K, 0:K], ynat[:, K * g:K * g + K],
                                 ynat[:, K * g:K * g + K], start=True, stop=True)
                nc.vector.tensor_copy(tBdYPY[K * g:K * g + K, K * g:K * g + K],
                                      pf[0:K, 0:K])

            # ---------------- front: P0raw/g0raw from dense nfT ----------------
            P0a = btile("P0a", (128, Q), "S4")
            g0a = btile("g0a", (128, Q), "S5")
            nfT2 = [btile("nfT2_0", (128, Q), "S1"),
                    btile("nfT2_1", (128, Q), "S2")]
            for piece in range(2):
                nc.sync.dma_start(nfT2[piece][:],
                                  nfT_d.ap()[128 * piece:128 * piece + 128, :])
            for j in range(8):
                pB = psb.tile([128, 2048], f32, tag="pI", name="pP0")
                pP = pB[:, 0:CH]
                pG0 = pB[:, CH:2 * CH]
                for piece in range(2):
                    for gl in (0, 1):
                        g = 2 * piece + gl
                        rt = F * gl
                        cols = slice(CH * j, CH * j + CH)
                        rhs = nfT2[piece][rt:rt + F, cols]
                        pos = (rt, K * g)
                        nc.tensor.matmul(pP[K * g:K * g + K, :],
                                         tWUF[rt:rt + F, :], rhs,
                                         start=True, stop=True, tile_position=pos)
                        nc.tensor.matmul(pG0[K * g:K * g + K, :],
                                         tWg1F[rt:rt + F, :], rhs,
                                         start=True, stop=True, tile_position=pos)
                dst = slice(CH * j, CH * j + CH)
                nc.scalar.activation(P0a[:, dst], pP[:], AF.Identity)
                nc.vector.tensor_copy(g0a[:, dst], pG0[:])

            tGS = ctile("gstat", (1, 8))
            nc.vector.tensor_scalar_mul(tGS[:, 0:2], cout[:, 0:2], 1.0 / NTOT)
            nc.vector.tensor_mul(tGS[:, 2:3], tGS[:, 0:1], tGS[:, 0:1])
            nc.vector.tensor_sub(tGS[:, 2:3], tGS[:, 1:2], tGS[:, 2:3])
            nc.vector.tensor_scalar_add(tGS[:, 2:3], tGS[:, 2:3], EPS)
            nc.scalar.activation(tGS[:, 3:4], tGS[:, 2:3], AF.Sqrt)
            nc.vector.reciprocal(tGS[:, 4:5], tGS[:, 3:4])              # alpha
            nc.vector.tensor_mul(tGS[:, 5:6], tGS[:, 4:5], tGS[:, 0:1])
            nc.vector.tensor_scalar_mul(tGS[:, 5:6], tGS[:, 5:6], -1.0)  # -am
            nc.vector.tensor_mul(tGS[:, 6:7], tGS[:, 5:6], tColF[:, K:K + 1])
            tAB = ctile("alphab", (128, 3))
            pf = pchunk()
            for ii, cidx in [(0, 4), (1, 5), (2, 6)]:
                nc.tensor.matmul(pf[0:128, ii:ii + 1], tOnesRow[:],
                                 tGS[:, cidx:cidx + 1], start=True, stop=True)
            nc.vector.tensor_copy(tAB[:], pf[0:128, 0:3])
            tBiasP0 = ctile("biasP0", (128, 1))
            nc.vector.tensor_mul(tBiasP0[:], tColUrep[:], tAB[:, 1:2])
            # normalize in place: P0a = alpha*P0raw + biasP0 ; g0a likewise
            nc.scalar.activation(P0a[:], P0a[:], AF.Identity,
                                 bias=tBiasP0[:], scale=tAB[:, 0:1])
            nc.scalar.activation(g0a[:], g0a[:], AF.Identity,
                                 bias=tAB[:, 2:3], scale=tAB[:, 0:1])
            if debug:
                d = dout("Gs", (F, F + 1)); nc.sync.dma_start(d.ap(), tGs[:])
                d = dout("st", (1, 8)); nc.sync.dma_start(d.ap(), tST[:, 0:8])
                d = dout("P0a", (128, Q)); nc.sync.dma_start(d.ap(), P0a[:])
                d = dout("g0a", (128, Q)); nc.sync.dma_start(d.ap(), g0a[:])
                d = dout("gstat", (1, 7)); nc.sync.dma_start(d.ap(), tGS[:, 0:7])
            xp = btile("xp", (128, Q), "S6", f32r)
            nc.scalar.activation(xp[:], P0a[:], AF.Relu)
            if debug:
                d = dout("xp0", (128, Q)); nc.sync.dma_start(d.ap(), xp[:].bitcast(f32))

            # ---------------- layers 0,1 ----------------
            Bst = None
            for l in range(2):
                bdYT = [tBdYT0, tBdYT1]
                mx = ctile("mx%d" % l, (128, 12))
                # pass 1: I^T fills, row maxes
                for h in range(2):
                    for hq in range(2):
                        pI = psb.tile([128, 2048], f32, tag="pI", name="pI")
                        for cc in range(4):
                            cs = slice(2048 * hq + CH * cc, 2048 * hq + CH * cc + CH)
                            nc.tensor.matmul(pI[:, CH * cc:CH * cc + CH],
                                             bdYT[h][:], xp[:, cs],
                                             start=True, stop=True)
                        nc.vector.reduce_max(mx[:, 4 * h + hq:4 * h + hq + 1],
                                             pI[:], axis=AX.X)
                    nc.vector.tensor_max(mx[:, 4 * h + 2:4 * h + 3],
                                         mx[:, 4 * h:4 * h + 1],
                                         mx[:, 4 * h + 1:4 * h + 2])
                    nc.vector.tensor_scalar_mul(mx[:, 4 * h + 3:4 * h + 4],
                                                mx[:, 4 * h + 2:4 * h + 3], -1.0)
                # pass 2: regenerate I^T, exp -> E, accumulate S
                E = btile("E%d" % l, (128, 2 * Q), "S1", f32r)
                S = ctile("S%d" % l, (128, 6))
                for h in range(2):
                    for hq in range(2):
                        pI = psb.tile([128, 2048], f32, tag="pI", name="pI")
                        for cc in range(4):
                            cs = slice(2048 * hq + CH * cc, 2048 * hq + CH * cc + CH)
                            nc.tensor.matmul(pI[:, CH * cc:CH * cc + CH],
                                             bdYT[h][:], xp[:, cs],
                                             start=True, stop=True)
                        nc.scalar.activation(
                            E[:, Q * h + 2048 * hq:Q * h + 2048 * hq + 2048],
                            pI[:], AF.Exp, bias=mx[:, 4 * h + 3:4 * h + 4],
                            accum_out=S[:, 2 * h + hq:2 * h + hq + 1])
                rS = ctile("rS%d" % l, (128, 2))
                for h in range(2):
                    nc.vector.tensor_add(S[:, 4 + h:5 + h], S[:, 2 * h:2 * h + 1],
                                         S[:, 2 * h + 1:2 * h + 2])
                    nc.vector.reciprocal(rS[:, h:h + 1], S[:, 4 + h:5 + h])
                bdYtil0 = ctile("bdYtil0_%d" % l, (128, 128), f32r)
                bdYtil1 = ctile("bdYtil1_%d" % l, (128, 128), f32r)
                zfill(bdYtil0)
                zfill(bdYtil1)
                for g in range(GPC):
                    sl = slice(K * g, K * g + K)
                    nc.vector.tensor_scalar(bdYtil0[sl, sl], ynat[0:K, sl],
                                            rS[sl, 0:1], None, ALU.mult)
                    nc.vector.tensor_scalar(bdYtil1[sl, sl], ynat[K:2 * K, sl],
                                            rS[sl, 1:2], None, ALU.mult)
                # w = relu(A y_p)
                wt = btile("w%d" % l, (128, Q), "S7", f32r)
                for cc in range(8):
                    pW = pchunk("pW")
                    nc.tensor.matmul(pW[:], bdYtil0[:], E[:, CH * cc:CH * cc + CH],
                                     start=True, stop=False)
                    nc.tensor.matmul(pW[:], bdYtil1[:],
                                     E[:, Q + CH * cc:Q + CH * cc + CH],
                                     start=False, stop=True)
                    nc.scalar.activation(wt[:, CH * cc:CH * cc + CH], pW[:], AF.Relu)
                # zpre = zh + B@vg + g0a ; om = sigmoid(-(zpre+gb))
                zs = btile("zs%d" % l, (128, Q), "S2")
                for cc in range(8):
                    pZ = pchunk("pZ")
                    nc.tensor.matmul(pZ[:], tBdW2V[:], wt[:, CH * cc:CH * cc + CH],
                                     start=True, stop=(Bst is None))
                    if Bst is not None:
                        nc.tensor.matmul(pZ[:], tBdVG[:], Bst[:, CH * cc:CH * cc + CH],
                                         start=False, stop=True)
                    nc.vector.tensor_add(zs[:, CH * cc:CH * cc + CH], pZ[:],
                                         g0a[:, CH * cc:CH * cc + CH])
                om = btile("om%d" % l, (128, Q), "S8")
                nc.scalar.activation(om[:], zs[:], AF.Sigmoid,
                                     bias=tNgb[:], scale=-1.0)
                if debug:
                    d = dout("w%d" % l, (128, Q)); nc.sync.dma_start(d.ap(), wt[:].bitcast(f32))
                    d = dout("om%d" % l, (128, Q)); nc.sync.dma_start(d.ap(), om[:])
                    d = dout("S%d" % l, (128, 6)); nc.sync.dma_start(d.ap(), S[:])
                    d = dout("mx%d" % l, (128, 8)); nc.sync.dma_start(d.ap(), mx[:, 0:8])
                # B' = om*(B - w) + w   (B=0 at l=0 -> B1 = w - om*w)
                if Bst is None:
                    m1 = btile("m1", (128, Q), "S2")
                    nc.vector.tensor_mul(m1[:], om[:], wt[:])
                    B1 = btile("B1", (128, Q), "S3", f32r)
                    nc.vector.tensor_sub(B1[:], wt[:], m1[:])
                    Bst = B1
                else:
                    dd = btile("dd", (128, Q), "S2")
                    nc.vector.tensor_sub(dd[:], Bst[:], wt[:])
                    nc.vector.tensor_mul(dd[:], dd[:], om[:])
                    B2 = btile("B2", (128, Q), "S1", f32r)   # E is dead by now
                    nc.vector.tensor_add(B2[:], wt[:], dd[:])
                    Bst = B2
                # P0a *= om ; g0a *= om (g0a only needed for next layer's gate)
                nc.vector.tensor_mul(P0a[:], P0a[:], om[:])
                if l == 0:
                    nc.vector.tensor_mul(g0a[:], g0a[:], om[:])
                # x_p for layer l+1 = relu(P0a + B@VU)
                xpre = btile("xpre%d" % l, (128, Q), "S2")
                for cc in range(8):
                    pX = pchunk("pX")
                    nc.tensor.matmul(pX[:], tBdVU[:], Bst[:, CH * cc:CH * cc + CH],
                                     start=True, stop=True)
                    nc.vector.tensor_add(xpre[:, CH * cc:CH * cc + CH], pX[:],
                                         P0a[:, CH * cc:CH * cc + CH])
                xp = btile("xp_%d" % (l + 1), (128, Q), "S6", f32r)
                nc.scalar.activation(xp[:], xpre[:], AF.Relu)
                if debug:
                    d = dout("xp%d" % (l + 1), (128, Q))
                    nc.sync.dma_start(d.ap(), xp[:].bitcast(f32))

            # ---------------- final ----------------
            tmp = btile("tmp", (128, Q), "S2")
            for cc in range(8):
                pP = pchunk("pF")
                nc.tensor.matmul(pP[:], tBdYPY[:], xp[:, CH * cc:CH * cc + CH],
                                 start=True, stop=True)
                nc.vector.tensor_mul(tmp[:, CH * cc:CH * cc + CH], pP[:],
                                     xp[:, CH * cc:CH * cc + CH])
            sOut = btile("sOut", (128, Q), "S4")     # P0a dead now
            for cc in range(8):
                pS = pchunk("pS")
                nc.tensor.matmul(pS[:], tSumbd[:], tmp[:, CH * cc:CH * cc + CH],
                                 start=True, stop=True)
                nc.scalar.activation(sOut[:, CH * cc:CH * cc + CH], pS[:], AF.Sigmoid)
            for g in range(GPC):
                nc.sync.dma_start(out_d.ap()[g:g + 1, :],
                                  sOut[K * g:K * g + 1, :])

    nc.compile()
    return nc


def _get_program(debug=False):
    key = "nc_dbg" if debug else "nc"
    if key not in _CACHE:
        _CACHE[key] = _build(debug)
    return _CACHE[key]


def make_in_maps(inputs):
    nf = np.ascontiguousarray(np.asarray(inputs["node_feats"], np.float32))
    frag = np.ascontiguousarray(
        np.asarray(inputs["frag_emb"], np.float32).reshape(B_ALL * J, F))
    fragT = np.ascontiguousarray(frag.T)                    # [F, B*J]
    W = np.ascontiguousarray(np.asarray(inputs["W_in"], np.float32))
    b = np.asarray(inputs["b_in"], np.float32).reshape(C, 1)
    U = np.ascontiguousarray(np.asarray(inputs["U"], np.float32))
    V = np.ascontiguousarray(np.asarray(inputs["V"], np.float32))
    q = np.asarray(inputs["q"], np.float32).reshape(K, 1)
    gW = np.asarray(inputs["gate_W"], np.float32).reshape(2 * C)
    gb = np.asarray(inputs["gate_b"], np.float32).reshape(1)
    in_maps = []
    for c in range(NC_):
        nfc = nf[c * NLOC:(c + 1) * NLOC]                   # [16384, F]
        g4 = nfc.reshape(GPC, NPG, F)                       # [4, 4096, F]
        # nfT: piece p = [g=2p ; g=2p+1] feature-major  -> [256, 4096]
        nfT = np.ascontiguousarray(
            g4.transpose(0, 2, 1).reshape(2, 128, NPG)).reshape(256, NPG)
        # nf65: piece p = 64 blocks of [128 nodes, F feats + 1.0]
        blk = nfc.reshape(2, 64, 128, F)                    # piece, blk, node, f
        nf65 = np.concatenate(
            [blk, np.ones((2, 64, 128, 1), np.float32)], axis=3)
        nf65 = np.ascontiguousarray(
            nf65.transpose(0, 2, 1, 3).reshape(256, 64 * (F + 1)))
        fragTl = np.ascontiguousarray(
            frag[c * GPC * J:(c + 1) * GPC * J].T)          # [F, 160]
        in_maps.append({
            "nfT": nfT,
            "nf65": nf65,
            "fragT": fragT,
            "fragTl": fragTl,
            "W_in": W, "b_in": b, "U": U, "V": V, "q": q,
            "gw1": np.ascontiguousarray(gW[:C].reshape(C, 1)),
            "gw2": np.ascontiguousarray(gW[C:].reshape(C, 1)),
            "gb": np.full((128, 1), gb[0], np.float32),
            "ident": np.eye(128, dtype=np.float32),
            "onesrow": np.ones((1, 128), np.float32),
        })
    return in_maps


def kernel(**inputs):
    from concourse.bass_utils import run_bass_kernel_spmd

    nc = _get_program()
    in_maps = make_in_maps(inputs)
    res = run_bass_kernel_spmd(nc, in_maps, core_ids=list(range(NC_)))
    return np.concatenate([r["out"] for r in res.results], axis=0)

